# revision 1
# baseline (speedup 1.0000x reference)
"""3-hop GNN message passing (BPR/LightGCN style) on 8 Trainium2 NeuronCores.

Strategy: destination-sharded SpMMs with hop fusion. The 5 logical
segment-sum SpMMs run as 3 edge passes:

  A: g1i = iu(embed_user)            gather fp32 eu rows (256B descriptors)
  B: (g1u, g2u) = ui([ei | g1i])     one gather pass over an interleaved
                                     bf16 pair table (256B rows, two hops
                                     per descriptor), two matmuls per chunk
                                     sharing one one-hot S
  C: (g2i, g3i) = iu([g1u/3|g2u/4])  same trick; both matmuls accumulate
                                     into one PSUM so the result is already
                                     1/3*g2i + 1/4*g3i

Each pass shards edges across cores by destination row, gathers source
rows with bulk dma_gather (int16 indices, 25000-row table sections),
applies per-edge weights via one-hot matmuls (S[e,slot]=w_e) on the tensor
engine, accumulates per-block PSUM windows, and scatter-adds finished rows
to HBM. AllGathers (bf16) assemble the interleaved tables between passes.
The final combine out = ei + 1/2 g1i + 1/3 g2i + 1/4 g3i is absorbed:
part_C is pre-initialized with ei + 0.5*g1i and pass C deposits the rest.
Gathers and scatters round-robin over multiple SWDGE queues (descriptor
throughput saturates around two queues per direction).
"""
import sys
import os

sys.path.insert(0, "/opt/trn_rl_repo")

import numpy as np
import ml_dtypes

import concourse.bass as bass
import concourse.bacc as bacc
import concourse.tile as tile
from concourse import bass_utils, mybir

# problem constants (hardcoded per harness contract)
U, I, D, E = 100000, 50000, 64, 1250000
NCORES = 8
DU = U // NCORES           # users per core (dest shard for ui-SpMMs)
DI = I // NCORES           # items per core (dest shard for iu-SpMMs)
SEC = 25000                # table section rows (int16 gather index range)
NSEC_IU = U // SEC         # 4 sections of the user-side tables
NSEC_UI = I // SEC         # 2 sections of the item-side tables
W = 64                     # dest window (one-hot slot count)
K = 128                    # edges per chunk (PE contraction dim)
CPB = 3                    # chunks per block
BPS = 16                   # blocks per super-block (= 2 PSUM banks)
CH_SB = BPS * CPB          # 48 chunks per super-block
IDX_SB = CH_SB * K         # 6144 gather indices per super-block
ROWS_SB = BPS * W          # 1024 scatter rows per super-block

BF16 = ml_dtypes.bfloat16

_LAST_RESULTS = None       # run metadata for test harness


def _pack_type(dest, src, w, dshard, nsec):
    """Pack edges for one SpMM type (iu or ui) into the uniform SPMD layout.

    Returns dict with per-core arrays (idx16, slot, w, sidx16) and NSB.
    """
    dest = dest.astype(np.int64)
    src = src.astype(np.int64)
    w = w.astype(np.float32)
    core_of = dest // dshard
    sec_of = src // SEC

    # per (core, section): lists of (slot_stream, src_stream, w_stream, blocks)
    per_cs = {}
    nblk_max = 0
    for c in range(NCORES):
        for s in range(nsec):
            m = (core_of == c) & (sec_of == s)
            d = dest[m] - c * dshard
            sl = src[m] - s * SEC
            wv = w[m]
            order = np.argsort(d, kind="stable")
            d, sl, wv = d[order], sl[order], wv[order]
            # unique dests in order with counts
            ud, ustart, ucnt = np.unique(d, return_index=True, return_counts=True)
            blocks = []   # (base, span, nedges)
            cur_base = -1
            cur_cnt = 0
            slot_arr = np.empty(len(d), np.float32)
            blk_of_edge = np.empty(len(d), np.int64)
            cap = CPB * K
            for t in range(len(ud)):
                u, st, k = int(ud[t]), int(ustart[t]), int(ucnt[t])
                if cur_base < 0 or (u - cur_base) >= W or (cur_cnt + k) > cap:
                    if cur_base >= 0:
                        blocks.append((cur_base, cur_span, cur_cnt))
                    cur_base = u
                    cur_cnt = 0
                cur_span = u - cur_base + 1
                slot_arr[st:st + k] = u - cur_base
                blk_of_edge[st:st + k] = len(blocks)
                cur_cnt += k
            if cur_base >= 0:
                blocks.append((cur_base, cur_span, cur_cnt))
            per_cs[(c, s)] = (d, sl, wv, slot_arr, blk_of_edge, blocks)
            nblk_max = max(nblk_max, len(blocks))

    nsb = (nblk_max + BPS - 1) // BPS
    nblk = nsb * BPS

    # emit per-core uniform arrays
    ncols_ch = nsec * nsb * CH_SB          # chunk columns total
    out = {
        "idx16": np.zeros((NCORES, 128, nsec * nsb * IDX_SB // 16), np.int16),
        "slot": np.zeros((NCORES, 128, ncols_ch), np.float32),
        "w": np.zeros((NCORES, 128, ncols_ch), np.float32),
        "sidx16": np.zeros((NCORES, 128, nsec * nsb * ROWS_SB // 16), np.int16),
        "nsb": nsb,
    }
    trash = dshard  # rows [dshard, dshard+W) are trash
    for c in range(NCORES):
        for s in range(nsec):
            d, sl, wv, slot_arr, blk_of_edge, blocks = per_cs[(c, s)]
            # stream arrays padded to nblk blocks
            slots_total = nblk * CPB * K
            idx_st = np.zeros(slots_total, np.int16)
            slot_st = np.zeros(slots_total, np.float32)
            w_st = np.zeros(slots_total, np.float32)
            # place each block's edges at block*cap
            if len(d):
                # edges are already grouped by block in order
                blk_change = np.r_[True, blk_of_edge[1:] != blk_of_edge[:-1]]
                grp_start = np.maximum.accumulate(
                    np.where(blk_change, np.arange(len(d)), 0))
                edge_pos_in_blk = np.arange(len(d)) - grp_start
                pos = blk_of_edge * (CPB * K) + edge_pos_in_blk
                idx_st[pos] = sl.astype(np.int16)
                slot_st[pos] = slot_arr
                w_st[pos] = wv
            # wrap into device layouts
            base_col = s * nsb  # super-block offset for this section
            # gather idx: position i -> (row i%16, col i//16), tiled 8x
            idxw = idx_st.reshape(-1, 16).T  # [16, slots/16]
            cw0 = base_col * (IDX_SB // 16)
            out["idx16"][c][:, cw0:cw0 + idxw.shape[1]] = np.tile(idxw, (8, 1))
            # slot/w: chunk-major [128, cols]
            sm = slot_st.reshape(-1, K).T    # [128, ncols_cs]
            wm = w_st.reshape(-1, K).T
            cc0 = base_col * CH_SB
            out["slot"][c][:, cc0:cc0 + sm.shape[1]] = sm
            out["w"][c][:, cc0:cc0 + wm.shape[1]] = wm
            # scatter rows: per super-block 1024 rows; row n -> p=n%128, j=n//128
            srows = np.full(nblk * W, trash, np.int64)
            rr = np.arange(nblk * W)
            srows += rr % W  # default trash + r (unique per slot)
            for b, (base, span, cnt) in enumerate(blocks):
                r = np.arange(span)
                srows[b * W + r[:span]] = base + r[:span]
            # reorder into scatter enumeration: for each sb: n in [0,1024):
            # p = n%128, j = n//128; block_local = j + 8*(p>=64); r = p%64
            n = np.arange(nsb * ROWS_SB)
            p = n % 128
            j = (n // 128) % 8
            sb_i = n // ROWS_SB
            blk_l = sb_i * BPS + j + 8 * (p >= 64)
            r = p % 64
            sidx_strm = srows[blk_l * W + r].astype(np.int16)
            sw = sidx_strm.reshape(-1, 16).T
            sc0 = base_col * (ROWS_SB // 16)
            out["sidx16"][c][:, sc0:sc0 + sw.shape[1]] = np.tile(sw, (8, 1))
    return out


def _build_program(nsb_iu, nsb_ui):
    nq = int(os.environ.get("KERNEL_NQ", "4"))
    nc = bacc.Bacc("TRN2", target_bir_lowering=False, debug=False,
                   num_devices=NCORES, num_swdge_queues=nq)
    f32 = mybir.dt.float32
    bf16 = mybir.dt.bfloat16
    i16 = mybir.dt.int16

    t_eu = nc.dram_tensor("embed_user", [U, D], f32, kind="ExternalInput")
    tb_init = nc.dram_tensor("tb_init", [DI + W, 2 * D], bf16,
                             kind="ExternalInput")
    ei_slice = nc.dram_tensor("ei_slice", [DI, D], f32, kind="ExternalInput")
    iota_in = nc.dram_tensor("iota", [128, W], f32, kind="ExternalInput")

    iu_cols = NSEC_IU * nsb_iu
    ui_cols = NSEC_UI * nsb_ui
    iu_idx = nc.dram_tensor("iu_idx", [128, iu_cols * IDX_SB // 16], i16, kind="ExternalInput")
    iu_slot = nc.dram_tensor("iu_slot", [128, iu_cols * CH_SB], f32, kind="ExternalInput")
    iu_w = nc.dram_tensor("iu_w", [128, iu_cols * CH_SB], f32, kind="ExternalInput")
    iu_sidx = nc.dram_tensor("iu_sidx", [128, iu_cols * ROWS_SB // 16], i16, kind="ExternalInput")
    ui_idx = nc.dram_tensor("ui_idx", [128, ui_cols * IDX_SB // 16], i16, kind="ExternalInput")
    ui_slot = nc.dram_tensor("ui_slot", [128, ui_cols * CH_SB], f32, kind="ExternalInput")
    ui_w = nc.dram_tensor("ui_w", [128, ui_cols * CH_SB], f32, kind="ExternalInput")
    ui_sidx = nc.dram_tensor("ui_sidx", [128, ui_cols * ROWS_SB // 16], i16, kind="ExternalInput")

    out_ext = nc.dram_tensor("out", [DI, D], f32, kind="ExternalOutput")

    tb_local = nc.dram_tensor("tb_local", [DI + W, 2 * D], bf16, kind="Internal")
    table_B = nc.dram_tensor("table_B", [I, 2 * D], bf16, kind="Internal")
    tc_local = nc.dram_tensor("tc_local", [DU + W, 2 * D], bf16, kind="Internal")
    table_C = nc.dram_tensor("table_C", [U, 2 * D], bf16, kind="Internal")
    part_C = nc.dram_tensor("part_C", [DI + W, D], f32, kind="Internal")

    rg = [list(range(NCORES))]
    stage = int(os.environ.get("KERNEL_STAGE", "0"))
    sub = int(os.environ.get("KERNEL_SUB", "3"))
    repeat = int(os.environ.get("KERNEL_REPEAT", "1"))

    with tile.TileContext(nc) as tc:
        with (
            tc.tile_pool(name="const", bufs=1) as cpool,
            tc.tile_pool(name="sb", bufs=4) as sb,
            tc.tile_pool(name="gp", bufs=3) as gp,
            tc.tile_pool(name="spool", bufs=6) as spool,
            tc.tile_pool(name="stgp", bufs=3) as stgp,
            tc.tile_pool(name="psum", bufs=2, space="PSUM") as pp,
            tc.tile_pool(name="psum2", bufs=2, space="PSUM") as pp2,
        ):
            iota_t = cpool.tile([128, W], f32)
            nc.sync.dma_start(out=iota_t[:], in_=iota_in[:])

            # zero tiles for clearing scatter targets
            ztb = cpool.tile([128, 8 * 2 * D], bf16)
            nc.vector.memset(ztb[:], 0.0)
            zf32 = cpool.tile([128, 8 * D], f32)
            nc.vector.memset(zf32[:], 0.0)

            def zero_t(part, nrows, width, zt):
                r0 = 0
                step = 128 * 8
                while r0 < nrows:
                    n = min(step, nrows - r0)
                    a = n // 128
                    if a >= 1:
                        nc.sync.dma_start(
                            out=part[r0:r0 + a * 128, :].rearrange(
                                "(a p) d -> p a d", p=128),
                            in_=zt[:, :a * width].rearrange(
                                "p (a d) -> p a d", a=a),
                        )
                        r0 += a * 128
                    else:
                        nc.sync.dma_start(out=part[r0:r0 + n, :],
                                          in_=zt[:n, :width])
                        r0 += n

            # tb_local <- tb_init (ei rows in cols 0:64, zeros elsewhere)
            nc.sync.dma_start(out=tb_local[:, :], in_=tb_init[:, :])
            zero_t(tc_local, DU + W, 2 * D, ztb)

            def spmm(kind, table, part, nsec, nsb, idx_in, slot_in, w_in,
                     sidx_in):
                """kind: 'A' (f32 single-table), 'B' (bf16 pair -> pair out),
                'C' (bf16 pair -> folded f32 out).

                Gathers span GSB=2 super-blocks; matmuls for B/C are one wide
                [128e -> 64slots x 128] op per chunk into [64, 512] PSUM tiles
                holding 4 wide block-columns each."""
                pair = kind in ("B", "C")
                gdt = bf16 if pair else f32
                gw = 2 * D if pair else D       # gathered row width (elems)
                sdt = bf16 if pair else f32     # S dtype must match gathered rows
                GSB = 2
                PREFETCH = int(os.environ.get("KERNEL_PF", "1"))
                FLUSH = int(os.environ.get("KERNEL_FLUSH", "4"))

                iters = []                      # (s, isb0, nsb_g)
                for s in range(nsec):
                    for isb0 in range(0, nsb, GSB):
                        iters.append((s, isb0, min(GSB, nsb - isb0)))

                def emit_gather(it_i):
                    s, isb0, nsb_g = iters[it_i]
                    g = s * nsb + isb0
                    qg = it_i % 2
                    idxt = sb.tile([128, GSB * IDX_SB // 16], i16, tag="idx")
                    nc.sync.dma_start(
                        out=idxt[:, :nsb_g * IDX_SB // 16],
                        in_=idx_in[:, g * (IDX_SB // 16):(g + nsb_g) * (IDX_SB // 16)])
                    slott = sb.tile([128, GSB * CH_SB], f32, tag="slot")
                    nc.sync.dma_start(
                        out=slott[:, :nsb_g * CH_SB],
                        in_=slot_in[:, g * CH_SB:(g + nsb_g) * CH_SB])
                    wt = sb.tile([128, GSB * CH_SB], f32, tag="w")
                    nc.sync.dma_start(
                        out=wt[:, :nsb_g * CH_SB],
                        in_=w_in[:, g * CH_SB:(g + nsb_g) * CH_SB])
                    sidxt = sb.tile([128, GSB * ROWS_SB // 16], i16, tag="sidx")
                    nc.sync.dma_start(
                        out=sidxt[:, :nsb_g * ROWS_SB // 16],
                        in_=sidx_in[:, g * (ROWS_SB // 16):(g + nsb_g) * (ROWS_SB // 16)])

                    gt = gp.tile([128, GSB * CH_SB * gw], gdt, tag="G")
                    if sub != 5:
                        nc.gpsimd.dma_gather(
                            out_ap=gt[:, :nsb_g * CH_SB * gw].rearrange(
                                "p (c d) -> p c d", c=nsb_g * CH_SB),
                            in_ap=table[s * SEC:(s + 1) * SEC, :],
                            idxs_ap=idxt[:, :nsb_g * IDX_SB // 16],
                            num_idxs=nsb_g * IDX_SB,
                            num_idxs_reg=nsb_g * IDX_SB,
                            elem_size=gw,
                            single_packet=False,
                            queue_num=qg,
                        )
                    return (gt, slott, wt, sidxt, nsb_g, qg)

                pend = {}
                defer = []                      # queued scatter emissions
                for it_i in range(len(iters)):
                    if it_i == 0:
                        for k in range(min(PREFETCH + 1, len(iters))):
                            pend[k] = emit_gather(k)
                    gt, slott, wt, sidxt, nsb_g, qg = pend.pop(it_i)
                    if sub == 0:
                        pass  # gathers only, unconsumed
                    else:
                        for isb_l in range(nsb_g):
                            _spmm_sb(kind, part, gt, slott, wt, sidxt,
                                     isb_l, (2 + len(defer) % 2) if nq >= 4
                                     else qg, sdt, gw, defer)
                    nxt = it_i + PREFETCH + 1
                    if nxt < len(iters):
                        pend[nxt] = emit_gather(nxt)
                    last = it_i + 1 == len(iters)
                    if len(defer) >= 2 * FLUSH or last or \
                            (it_i + 1 < len(iters) and
                             iters[it_i + 1][0] != iters[it_i][0]):
                        for fn in defer:
                            fn()
                        defer = []

            def _spmm_sb(kind, part, gt, slott, wt, sidxt, isb_l, qs, sdt, gw,
                         defer):
                c0 = isb_l * CH_SB             # chunk offset in gt/slott/wt
                pair = kind in ("B", "C")
                # PSUM tiles: A: psA/psB narrow (8 blk x 64); B/C: 4 wide
                # tiles of 4 blocks x 128 cols
                if kind == "A":
                    psA = pp.tile([64, 512], f32, tag="psA")
                    psB = pp.tile([64, 512], f32, tag="psB")
                    pst = [psA, psB]
                else:
                    pst = [pp.tile([64, 512], f32, tag="psA", name="psA"),
                           pp.tile([64, 512], f32, tag="psB", name="psB"),
                           pp2.tile([64, 512], f32, tag="psC", name="psC"),
                           pp2.tile([64, 512], f32, tag="psD", name="psD")]
                for blk in range(BPS):
                    for ch in range(CPB):
                        ci = c0 + blk * CPB + ch
                        st = spool.tile([128, W], sdt, tag="S")
                        nc.vector.tensor_scalar(
                            out=st[:],
                            in0=iota_t[:],
                            scalar1=slott[:, ci:ci + 1],
                            scalar2=wt[:, ci:ci + 1],
                            op0=mybir.AluOpType.is_equal,
                            op1=mybir.AluOpType.mult,
                        )
                        if kind == "A":
                            p1 = pst[blk // 8]
                            col = blk % 8
                            nc.tensor.matmul(
                                out=p1[:, col * D:(col + 1) * D],
                                lhsT=st[:],
                                rhs=gt[:, ci * D:(ci + 1) * D],
                                start=(ch == 0),
                                stop=(ch == CPB - 1),
                            )
                        else:  # wide: [64 slots, 128] per block
                            p1 = pst[blk // 4]
                            col = blk % 4
                            nc.tensor.matmul(
                                out=p1[:, col * 2 * D:(col + 1) * 2 * D],
                                lhsT=st[:],
                                rhs=gt[:, ci * 2 * D:(ci + 1) * 2 * D],
                                start=(ch == 0),
                                stop=(ch == CPB - 1),
                            )
                # PSUM -> staging -> scatter
                sx = sidxt[:, isb_l * (ROWS_SB // 16):(isb_l + 1) * (ROWS_SB // 16)]
                if kind == "A":
                    # stg rows [0:64]=0 (ei half), [64:128]=g1i bf16
                    stg = stgp.tile([128, 8 * 2 * D], bf16, tag="stgA", bufs=10)
                    nc.vector.memset(stg[:], 0.0)
                    sv = stg[:].rearrange("p (c d) -> p c d", c=8)
                    nc.scalar.activation(
                        out=sv[0:64, :, D:2 * D], in_=pst[0][:].rearrange(
                            "p (c d) -> p c d", c=8),
                        func=mybir.ActivationFunctionType.Copy)
                    nc.scalar.activation(
                        out=sv[64:128, :, D:2 * D], in_=pst[1][:].rearrange(
                            "p (c d) -> p c d", c=8),
                        func=mybir.ActivationFunctionType.Copy)
                    if sub <= 1:
                        nc.gpsimd.dma_start(out=part[0:128, :],
                                            in_=stg[:, 0:2 * D])
                        return
                    src = (ztb[:, :8 * 2 * D].rearrange("p (c d) -> p c d", c=8)
                           if sub == 2 else sv)
                    defer.append(lambda src=src, sx=sx, qs=qs:
                                 nc.gpsimd.dma_scatter_add(
                                     part[:], src, sx, ROWS_SB, ROWS_SB, 2 * D,
                                     queue_num=qs))
                elif kind == "B":
                    # stg rows = [g1u/3 | g2u/4] bf16; wide psum tiles hold
                    # 4 blocks x [g1u | g2u] each
                    stg = stgp.tile([128, 8 * 2 * D], bf16, tag="stgB", bufs=10)
                    sv = stg[:].rearrange("p (c d) -> p c d", c=8)
                    for half in range(2):          # stg partition half
                        po = half * 64
                        for jt in range(2):        # stg block cols j<4 / j>=4
                            t = pst[half * 2 + jt]
                            tv = t[:].rearrange("p (c d) -> p c d", c=4)
                            nc.scalar.activation(
                                out=sv[po:po + 64, jt * 4:(jt + 1) * 4, 0:D],
                                in_=tv[:, :, 0:D],
                                func=mybir.ActivationFunctionType.Copy,
                                scale=1.0 / 3.0)
                            nc.scalar.activation(
                                out=sv[po:po + 64, jt * 4:(jt + 1) * 4, D:2 * D],
                                in_=tv[:, :, D:2 * D],
                                func=mybir.ActivationFunctionType.Copy,
                                scale=0.25)
                    if sub <= 1:
                        nc.gpsimd.dma_start(out=part[0:128, :],
                                            in_=stg[:, 0:2 * D])
                        return
                    src = (ztb[:, :8 * 2 * D].rearrange("p (c d) -> p c d", c=8)
                           if sub == 2 else sv)
                    defer.append(lambda src=src, sx=sx, qs=qs:
                                 nc.gpsimd.dma_scatter_add(
                                     part[:], src, sx, ROWS_SB, ROWS_SB, 2 * D,
                                     queue_num=qs))
                else:  # C: f32 folded rows (sum of wide halves)
                    stg = stgp.tile([128, 8 * D], f32, tag="stgC", bufs=10)
                    sv = stg[:].rearrange("p (c d) -> p c d", c=8)
                    for half in range(2):
                        po = half * 64
                        for jt in range(2):
                            t = pst[half * 2 + jt]
                            tmp = stgp.tile([64, 512], f32, tag="ctmp")
                            nc.scalar.activation(
                                out=tmp[:], in_=t[:],
                                func=mybir.ActivationFunctionType.Copy)
                            tv = tmp[:].rearrange("p (c d) -> p c d", c=4)
                            nc.vector.tensor_add(
                                sv[po:po + 64, jt * 4:(jt + 1) * 4, :],
                                tv[:, :, 0:D], tv[:, :, D:2 * D])
                    if sub <= 1:
                        nc.gpsimd.dma_start(out=part[0:128, :D],
                                            in_=stg[:, 0:D])
                        return
                    src = (zf32[:, :8 * D].rearrange("p (c d) -> p c d", c=8)
                           if sub == 2 else sv)
                    defer.append(lambda src=src, sx=sx, qs=qs:
                                 nc.gpsimd.dma_scatter_add(
                                     part[:], src, sx, ROWS_SB, ROWS_SB, D,
                                     queue_num=qs))

            def init_part_c():
                # part_C[0:DI] = ei + 0.5 * g1i  (g1i = tb_local[:, D:2D] bf16)
                step = 128 * 24
                r0 = 0
                while r0 < DI:
                    n = min(step, DI - r0)
                    a = max(n // 128, 1)
                    n = a * 128 if n >= 128 else n
                    if n >= 128:
                        view = lambda t, w0, w1: t[r0:r0 + n, w0:w1].rearrange(
                            "(a p) d -> p a d", p=128)
                        eit = sb.tile([128, a * D], f32, tag="pc_e", bufs=2)
                        g1t = sb.tile([128, a * D], bf16, tag="pc_g", bufs=2)
                        acc = sb.tile([128, a * D], f32, tag="pc_a", bufs=2)
                        nc.sync.dma_start(
                            out=eit[:].rearrange("p (a d) -> p a d", a=a),
                            in_=view(ei_slice, 0, D))
                        nc.sync.dma_start(
                            out=g1t[:].rearrange("p (a d) -> p a d", a=a),
                            in_=view(tb_local, D, 2 * D))
                        nc.vector.tensor_scalar_mul(acc[:], g1t[:], 0.5)
                        nc.vector.tensor_add(acc[:], acc[:], eit[:])
                        nc.sync.dma_start(
                            out=view(part_C, 0, D),
                            in_=acc[:].rearrange("p (a d) -> p a d", a=a))
                        r0 += n
                    else:
                        eit = sb.tile([128, D], f32, tag="pc_e", bufs=2)
                        g1t = sb.tile([128, D], bf16, tag="pc_g", bufs=2)
                        acc = sb.tile([128, D], f32, tag="pc_a", bufs=2)
                        nc.sync.dma_start(out=eit[:n], in_=ei_slice[r0:r0 + n, :])
                        nc.sync.dma_start(out=g1t[:n], in_=tb_local[r0:r0 + n, D:2 * D])
                        nc.vector.tensor_scalar_mul(acc[:n], g1t[:n], 0.5)
                        nc.vector.tensor_add(acc[:n], acc[:n], eit[:n])
                        nc.sync.dma_start(out=part_C[r0:r0 + n, :], in_=acc[:n])
                        r0 += n
                # trash rows of part_C can hold garbage (never read), but
                # scatter-add needs them initialized to avoid NaN poisoning
                zf = cpool.tile([128, D], f32)
                nc.vector.memset(zf[:], 0.0)
                nc.sync.dma_start(out=part_C[DI:DI + W, :], in_=zf[:W])

            def merge_parts(dst, srcs, nrows, w0, w1, dt, accum_dst=False,
                            dst_out=None, copy_only=False):
                """dst_out[r, w0:w1] = (dst if accum_dst) + sum(srcs) over
                the same slice. Chunked [128 x a*(w1-w0)] DVE adds."""
                wid = w1 - w0
                step = 128 * 8
                r0 = 0
                while r0 < nrows:
                    n = min(step, nrows - r0)
                    a = n // 128
                    full = a >= 1
                    if full:
                        n = a * 128
                    else:
                        a = 1
                    view = lambda t: (
                        t[r0:r0 + n, w0:w1].rearrange("(a p) d -> p a d", p=128)
                        if full else t[r0:r0 + n, w0:w1])
                    tiles = []
                    for si, s_t in enumerate(srcs):
                        tt = sb.tile([128, a * wid], dt, tag=f"mg{si}", bufs=2,
                                     name=f"mg{si}")
                        if full:
                            nc.sync.dma_start(
                                out=tt[:].rearrange("p (a d) -> p a d", a=a),
                                in_=view(s_t))
                        else:
                            nc.sync.dma_start(out=tt[:n, :wid], in_=view(s_t))
                        tiles.append(tt)
                    acc = sb.tile([128, a * wid], dt, tag="mga", bufs=2)
                    pa = ((slice(0, 128), slice(0, a * wid)) if full else
                          (slice(0, n), slice(0, wid)))
                    if copy_only:
                        acc = tiles[0]
                        rest = []
                    elif accum_dst:
                        dt0 = sb.tile([128, a * wid], dt, tag="mgd", bufs=2)
                        if full:
                            nc.sync.dma_start(
                                out=dt0[:].rearrange("p (a d) -> p a d", a=a),
                                in_=view(dst))
                        else:
                            nc.sync.dma_start(out=dt0[:n, :wid], in_=view(dst))
                        nc.vector.tensor_add(acc[pa], dt0[pa], tiles[0][pa])
                        rest = tiles[1:]
                    else:
                        nc.vector.tensor_add(acc[pa], tiles[0][pa],
                                             tiles[1][pa])
                        rest = tiles[2:]
                    for tt in rest:
                        nc.vector.tensor_add(acc[pa], acc[pa], tt[pa])
                    tgt = dst_out if dst_out is not None else dst
                    if full:
                        nc.sync.dma_start(
                            out=tgt[r0:r0 + n, w0:w1].rearrange(
                                "(a p) d -> p a d", p=128),
                            in_=acc[:].rearrange("p (a d) -> p a d", a=a))
                    else:
                        nc.sync.dma_start(out=tgt[r0:r0 + n, w0:w1],
                                          in_=acc[:n, :wid])
                    r0 += n

            for _rep in range(repeat):
                # pass A: g1i into per-section tbp
                spmm("A", t_eu, tb_local, NSEC_IU, nsb_iu,
                     iu_idx, iu_slot, iu_w, iu_sidx)
                if stage == 1:
                    continue
                if stage != 2:
                    nc.gpsimd.collective_compute(
                        "AllGather", mybir.AluOpType.bypass, replica_groups=rg,
                        ins=[tb_local[0:DI, :]], outs=[table_B[:]])
                init_part_c()
                # pass B: (g1u/3 | g2u/4) into per-section tcp
                spmm("B", table_B, tc_local, NSEC_UI, nsb_ui,
                     ui_idx, ui_slot, ui_w, ui_sidx)
                if stage != 2:
                    nc.gpsimd.collective_compute(
                        "AllGather", mybir.AluOpType.bypass, replica_groups=rg,
                        ins=[tc_local[0:DU, :]], outs=[table_C[:]])
                # pass C: 1/3 g2i + 1/4 g3i scatter-added onto pcp
                if stage != 4:
                    spmm("C", table_C, part_C, NSEC_IU, nsb_iu,
                         iu_idx, iu_slot, iu_w, iu_sidx)

            # out = part_C[0:DI]
            merge_parts(part_C, [part_C], DI, 0, D, f32, dst_out=out_ext,
                        copy_only=True)

    nc.compile()
    return nc


def _make_in_maps(inputs, iu, ui):
    embed_user = np.asarray(inputs["embed_user"], np.float32)
    embed_item = np.asarray(inputs["embed_item"], np.float32)
    iota = np.broadcast_to(np.arange(W, dtype=np.float32), (128, W)).copy()
    in_maps = []
    for c in range(NCORES):
        ei_sl = np.ascontiguousarray(embed_item[c * DI:(c + 1) * DI])
        tb0 = np.zeros((DI + W, 2 * D), BF16)
        tb0[:DI, :D] = ei_sl.astype(BF16)
        in_maps.append({
            "embed_user": embed_user,
            "tb_init": tb0,
            "ei_slice": ei_sl,
            "iota": iota,
            "iu_idx": iu["idx16"][c], "iu_slot": iu["slot"][c],
            "iu_w": iu["w"][c], "iu_sidx": iu["sidx16"][c],
            "ui_idx": ui["idx16"][c], "ui_slot": ui["slot"][c],
            "ui_w": ui["w"][c], "ui_sidx": ui["sidx16"][c],
        })
    return in_maps


def kernel(embed_user, embed_item, edge_vals, u_idx, i_idx):
    global _LAST_RESULTS
    inputs = {
        "embed_user": np.asarray(embed_user, np.float32),
        "embed_item": np.asarray(embed_item, np.float32),
    }
    edge_vals = np.asarray(edge_vals, np.float32)
    u_idx = np.asarray(u_idx).astype(np.int64)
    i_idx = np.asarray(i_idx).astype(np.int64)

    # pack both SpMM edge types
    iu = _pack_type(i_idx, u_idx, edge_vals, DI, NSEC_IU)   # dest=item, src=user
    ui = _pack_type(u_idx, i_idx, edge_vals, DU, NSEC_UI)   # dest=user, src=item

    nc = _build_program(iu["nsb"], ui["nsb"])
    in_maps = _make_in_maps(inputs, iu, ui)

    trace = bool(int(os.environ.get("KERNEL_TRACE", "0")))
    res = bass_utils.run_bass_kernel_spmd(
        nc, in_maps, core_ids=list(range(NCORES)), trace=trace)
    _LAST_RESULTS = res
    out = np.concatenate([res.results[c]["out"] for c in range(NCORES)], axis=0)
    return out



# revision 4
# speedup vs baseline: 3.8462x; 3.8462x over previous
"""3-hop GNN message passing (LightGCN style) on 8 Trainium2 NeuronCores.

v4 dataflow (HBM pair-table gathers on 4 SWDGE queues, host-streamed one-hot
S, PE window matmuls, DVE accumulation in SBUF f32, no scatter DMA) with
v4's low-pad sectioning (iu: 4 src sections, ui: 2) PLUS collective overlap:
destination windows are processed half-by-half, and each half's AllGather
launches as soon as that half of the accumulator is final (transfers overlap
the second half's compute; the trailing half-collective is smaller).  Tables
are rank-major (row r of core c at c*R*128 + j*128 + p) so a rank-half is a
contiguous input; the split AllGathers write strided views of one table.
"""
import os
import sys

sys.path.insert(0, "/opt/trn_rl_repo")

import numpy as np
import ml_dtypes

import concourse.bass as bass
import concourse.bacc as bacc
import concourse.tile as tile
from concourse import bass_utils, mybir

U, I, D, E = 100000, 50000, 64, 1250000
NCORES = 8
DU = U // NCORES
DI = I // NCORES
RU = 98
RI = 49
NWU, NWI = 98, 49
UH0, UH1 = 49, 49           # user rank halves
IH0, IH1 = 25, 24           # item rank halves
K = 128
GCH = int(os.environ.get("KERNEL_GCH", "24"))
SECU = 2 * RU * 128         # 25088 user rows per section (core pair)
SECI = 4 * RI * 128         # 25088 item rows per section (core quad)

BF16 = ml_dtypes.bfloat16

_LAST_RESULTS = None


def _pack_v5(dest, src, wv, dshard, nwin, dh_w0, nsec, sec_a, idxl_a):
    """Half-ordered aligned-window pack. Chunk order: (dest_half, sec, win)."""
    dest = np.asarray(dest, np.int64)
    wv = np.asarray(wv, np.float32)
    core = dest // dshard
    dl = dest - core * dshard
    win = dl >> 7
    slot = dl & 127
    dh = (win >= dh_w0).astype(np.int64)
    okey = ((core * 2 + dh) * nsec + sec_a) * nwin + win
    order = np.argsort(okey, kind="stable")
    g = okey[order]
    idx_s = idxl_a[order].astype(np.int16)
    slot_s = slot[order]
    w_s = wv[order]
    uniq, start, cnt = np.unique(g, return_index=True, return_counts=True)
    cnts = np.zeros(NCORES * 2 * nsec * nwin, np.int64)
    cnts[uniq] = cnt
    cnts = cnts.reshape(NCORES, 2 * nsec * nwin)
    nch_flat = np.ceil(cnts.max(axis=0) / K).astype(np.int64)
    nch = nch_flat.reshape(2, nsec, nwin)
    for hh in range(2):
        wl = np.arange(nwin)
        in_h = (wl >= dh_w0) == (hh == 1)
        assert (nch[hh][:, in_h] >= 1).all(), "zero-chunk window"
        assert (nch[hh][:, ~in_h] == 0).all()
    ch_off = np.zeros(2 * nsec * nwin + 1, np.int64)
    ch_off[1:] = np.cumsum(nch_flat)
    total_ch = int(ch_off[-1])

    q = np.arange(len(g)) - np.repeat(start, cnt)
    sw_key = g % (2 * nsec * nwin)
    ch_of = ch_off[sw_key] + q // K
    lane = q % K
    c_of = g // (2 * nsec * nwin)

    idx_st = np.zeros((NCORES, total_ch * K), np.int16)
    s_st = np.zeros((NCORES, 128, total_ch * 128), BF16)
    idx_st[c_of, ch_of * K + lane] = idx_s
    s_st[c_of, lane, ch_of * 128 + slot_s] = w_s.astype(BF16)

    idx16 = np.zeros((NCORES, 128, total_ch * 8), np.int16)
    for c in range(NCORES):
        wrap = idx_st[c].reshape(-1, 16).T
        idx16[c] = np.tile(wrap, (8, 1))
    return {"nch": nch, "total_ch": total_ch, "idx16": idx16, "S": s_st}


def _pack_all(inputs):
    edge_vals = np.asarray(inputs["edge_vals"], np.float32)
    u_idx = np.asarray(inputs["u_idx"]).astype(np.int64)
    i_idx = np.asarray(inputs["i_idx"]).astype(np.int64)

    # iu: dest=item windows (halves at 25), src=user rows, 4 pair-sections
    c = u_idx // DU
    rl = u_idx % DU
    j, p = rl // 128, rl % 128
    sec_u = u_idx // 25000
    idxl_u = (c % 2) * (RU * 128) + j * 128 + p
    iu = _pack_v5(i_idx, u_idx, edge_vals, DI, NWI, IH0, 4, sec_u, idxl_u)

    # ui: dest=user windows (halves at 49), src=item rows, 2 quad-sections
    c = i_idx // DI
    rl = i_idx % DI
    j, p = rl // 128, rl % 128
    sec_i = i_idx // 25000
    idxl_i = (c % 4) * (RI * 128) + j * 128 + p
    ui = _pack_v5(u_idx, i_idx, edge_vals, DU, NWU, UH0, 2, sec_i, idxl_i)
    return iu, ui


def _build_v2(iu_nch, ui_nch):
    nq = 4
    nc = bacc.Bacc("TRN2", target_bir_lowering=False, debug=False,
                   num_devices=NCORES, num_swdge_queues=nq)
    f32 = mybir.dt.float32
    bf16 = mybir.dt.bfloat16
    i16 = mybir.dt.int16

    iu_ch = int(iu_nch.sum())
    ui_ch = int(ui_nch.sum())

    eu_tbl = nc.dram_tensor("eu_tbl", [8 * RU * 128, 128], bf16,
                            kind="ExternalInput")
    ei_w = nc.dram_tensor("ei_w", [128, RI * 128], bf16, kind="ExternalInput")
    ei_f32w = nc.dram_tensor("ei_f32w", [128, RI * 64], f32,
                             kind="ExternalInput")
    iu_idx = nc.dram_tensor("iu_idx", [128, iu_ch * 8], i16, kind="ExternalInput")
    iu_S = nc.dram_tensor("iu_S", [128, iu_ch * 128], bf16, kind="ExternalInput")
    ui_idx = nc.dram_tensor("ui_idx", [128, ui_ch * 8], i16, kind="ExternalInput")
    ui_S = nc.dram_tensor("ui_S", [128, ui_ch * 128], bf16, kind="ExternalInput")

    out_ext = nc.dram_tensor("out", [DI, D], f32, kind="ExternalOutput")

    tbw_h0 = nc.dram_tensor("tbw_h0", [IH0 * 128, 128], bf16, kind="Internal")
    tbw_h1 = nc.dram_tensor("tbw_h1", [IH1 * 128, 128], bf16, kind="Internal")
    table_B = nc.dram_tensor("table_B", [8 * RI * 128, 128], bf16,
                             kind="Internal")
    gBh0 = nc.dram_tensor("gBh0", [8 * IH0 * 128, 128], bf16,
                          kind="Internal", addr_space="Shared")
    gBh1 = nc.dram_tensor("gBh1", [8 * IH1 * 128, 128], bf16,
                          kind="Internal", addr_space="Shared")
    tcw_h0 = nc.dram_tensor("tcw_h0", [UH0 * 128, 128], bf16, kind="Internal")
    tcw_h1 = nc.dram_tensor("tcw_h1", [UH1 * 128, 128], bf16, kind="Internal")
    table_C = nc.dram_tensor("table_C", [8 * RU * 128, 128], bf16,
                             kind="Internal")
    gCh0 = nc.dram_tensor("gCh0", [8 * UH0 * 128, 128], bf16,
                          kind="Internal", addr_space="Shared")
    gCh1 = nc.dram_tensor("gCh1", [8 * UH1 * 128, 128], bf16,
                          kind="Internal", addr_space="Shared")

    rg = [list(range(NCORES))]
    stage = int(os.environ.get("KERNEL_STAGE", "0"))
    sub = int(os.environ.get("KERNEL_SUB", "3"))
    repeat = int(os.environ.get("KERNEL_REPEAT", "1"))

    with tile.TileContext(nc) as tc:
        with (
            tc.tile_pool(name="const", bufs=1) as cpool,
            tc.tile_pool(name="sstream",
                         bufs=int(os.environ.get("KERNEL_SSB", "6"))) as sstream,
            tc.tile_pool(name="gp",
                         bufs=int(os.environ.get("KERNEL_GPB", "7"))) as gp,
            tc.tile_pool(name="sp",
                         bufs=int(os.environ.get("KERNEL_SPB", "6"))) as sp,
            tc.tile_pool(name="psum", bufs=int(os.environ.get("KERNEL_PSB", "6")),
                         space="PSUM") as pp,
        ):
            accA = cpool.tile([128, RI * 128], f32)
            accB = cpool.tile([128, RU * 128], f32)
            accC = cpool.tile([128, RI * 64], f32)
            stg = cpool.tile([128, RU * 128], bf16)
            eif = cpool.tile([128, RI * 64], f32)

            def spmm(slice_fn, nsec, nwin, dh_w0, nch, idx_in, S_in,
                     accum, fold, h0_hook):
                ch0 = [0]
                qi = [0]

                def do_half(dhh):
                    wins = (list(range(dh_w0)) if dhh == 0
                            else list(range(dh_w0, nwin)))
                    quads = []
                    b = 0
                    while b < len(wins):
                        quads.append((wins[b], min(4, len(wins) - b)))
                        b += 4
                    q_of_win = {}
                    for (w0q, qn) in quads:
                        for w in range(w0q, w0q + qn):
                            q_of_win[w] = (w0q, qn)
                    for s in range(nsec):
                        counts = [int(nch[dhh][s][w]) for w in wins]
                        sec_ch = sum(counts)
                        groups = []
                        b2 = 0
                        while b2 < sec_ch:
                            groups.append((ch0[0] + b2, min(GCH, sec_ch - b2)))
                            b2 += GCH
                        tslice = slice_fn(s)
                        win_of = np.repeat(wins, counts)
                        kpos = np.concatenate([np.arange(n) for n in counts])

                        def emit_group(gi):
                            gb, gl = groups[gi]
                            idxt = sstream.tile([128, GCH * 8], i16,
                                                tag="idx", name="idxt")
                            nc.sync.dma_start(
                                out=idxt[:, :gl * 8],
                                in_=idx_in[:, gb * 8:(gb + gl) * 8])
                            st = sp.tile([128, GCH * 128], bf16, tag="S",
                                         name="st")
                            seng = (nc.scalar if os.environ.get(
                                "KERNEL_SLENG", "sync") == "act" else nc.sync)
                            seng.dma_start(
                                out=st[:, :gl * 128],
                                in_=S_in[:, gb * 128:(gb + gl) * 128])
                            gt = gp.tile([128, GCH * K], bf16, tag="G",
                                         name="gt")
                            if sub != 5:
                                nc.gpsimd.dma_gather(
                                    out_ap=gt[:, :gl * K].rearrange(
                                        "p (c d) -> p c d", c=gl),
                                    in_ap=tslice,
                                    idxs_ap=idxt[:, :gl * 8],
                                    num_idxs=gl * K,
                                    num_idxs_reg=gl * K,
                                    elem_size=128,
                                    single_packet=False,
                                    queue_num=qi[0] % nq,
                                )
                                qi[0] += 1
                            return (gt, st, gb, gl)

                        pend = {}
                        cur_ps = [None]
                        pending_add = []

                        def flush_adds():
                            for fn in pending_add:
                                fn()
                            pending_add.clear()

                        pf = int(os.environ.get("KERNEL_PF", "4"))
                        for gi in range(len(groups)):
                            if gi == 0:
                                for kk2 in range(min(pf, len(groups))):
                                    pend[kk2] = emit_group(kk2)
                            gt, st, gb, gl = pend.pop(gi)
                            nxt = gi + pf
                            if nxt < len(groups):
                                pend[nxt] = emit_group(nxt)
                            if sub == 0:
                                continue
                            flush_adds()
                            for lci in range(gl):
                                ci = gb + lci - ch0[0]
                                w0 = int(win_of[ci])
                                kk = int(kpos[ci])
                                nw = int(nch[dhh][s][w0])
                                w0q, qn = q_of_win[w0]
                                col = w0 - w0q
                                if kk == 0 and col == 0:
                                    cur_ps[0] = pp.tile([128, 512], f32,
                                                        tag="ps", name="ps")
                                ps = cur_ps[0]
                                nc.tensor.matmul(
                                    out=ps[:, col * 128:(col + 1) * 128],
                                    lhsT=st[:, lci * 128:(lci + 1) * 128],
                                    rhs=gt[:, lci * K:(lci + 1) * K],
                                    start=(kk == 0),
                                    stop=(kk == nw - 1),
                                )
                                if kk == nw - 1 and col == qn - 1:
                                    if sub == 2:
                                        continue
                                    if fold:
                                        def add_fn(ps=ps, w0q=w0q, qn=qn):
                                            sl = accum[
                                                :, w0q * 64:(w0q + qn) * 64
                                            ].rearrange(
                                                "p (q f) -> p q f", q=qn)
                                            pv = ps[:, :qn * 128].rearrange(
                                                "p (q f) -> p q f", q=qn)
                                            nc.vector.tensor_add(
                                                sl, sl, pv[:, :, 0:64])
                                            nc.vector.tensor_add(
                                                sl, sl, pv[:, :, 64:128])
                                    else:
                                        def add_fn(ps=ps, w0q=w0q, qn=qn):
                                            sl = accum[:, w0q * 128:
                                                       (w0q + qn) * 128]
                                            nc.vector.tensor_add(
                                                sl, sl, ps[:, :qn * 128])
                                    pending_add.append(add_fn)
                        flush_adds()
                        ch0[0] += sec_ch

                do_half(0)
                if h0_hook is not None:
                    h0_hook()
                do_half(1)

            def cast_write_gather(acc_ap, ranks, tw, gtbl, tbl_view, scales):
                if scales is None:
                    nc.scalar.activation(
                        out=stg[:, :ranks * 128], in_=acc_ap,
                        func=mybir.ActivationFunctionType.Copy)
                else:
                    av = acc_ap.rearrange("p (j f) -> p j f", j=ranks)
                    sv = stg[:, :ranks * 128].rearrange(
                        "p (j f) -> p j f", j=ranks)
                    nc.scalar.activation(
                        out=sv[:, :, 0:64], in_=av[:, :, 0:64],
                        func=mybir.ActivationFunctionType.Copy,
                        scale=scales[0])
                    nc.scalar.activation(
                        out=sv[:, :, 64:128], in_=av[:, :, 64:128],
                        func=mybir.ActivationFunctionType.Copy,
                        scale=scales[1])
                nc.sync.dma_start(
                    out=tw[:, :].rearrange("(j p) d -> p j d", p=128),
                    in_=stg[:, :ranks * 128].rearrange(
                        "p (j f) -> p j f", j=ranks))
                if stage != 2:
                    nc.gpsimd.collective_compute(
                        "AllGather", mybir.AluOpType.bypass,
                        replica_groups=rg, ins=[tw[:, :]], outs=[gtbl[:, :]])
                # merge the gathered half into the unified rank-major table
                nc.sync.dma_start(
                    out=tbl_view,
                    in_=gtbl[:, :].rearrange("(c r) d -> c r d", c=8))

            def sl_A(s):
                return eu_tbl[s * SECU:(s + 1) * SECU, :]

            def sl_B(s):
                return table_B[s * SECI:(s + 1) * SECI, :]

            def sl_C(s):
                return table_C[s * SECU:(s + 1) * SECU, :]

            tB_c = table_B[:, :].rearrange("(c r) d -> c r d", c=8)
            tC_c = table_C[:, :].rearrange("(c r) d -> c r d", c=8)

            for _rep in range(repeat):
                nc.sync.dma_start(out=stg[:, :RI * 128], in_=ei_w[:, :])
                nc.vector.tensor_copy(accA[:], stg[:, :RI * 128])
                nc.vector.memset(accB[:], 0.0)
                nc.sync.dma_start(out=eif[:], in_=ei_f32w[:, :])

                def hookA0():
                    cast_write_gather(accA[:, :IH0 * 128], IH0, tbw_h0,
                                      gBh0, tB_c[:, 0:IH0 * 128, :], None)
                spmm(sl_A, 4, NWI, IH0, iu_nch, iu_idx, iu_S, accA,
                     False, hookA0)
                if stage == 1:
                    continue
                cast_write_gather(accA[:, IH0 * 128:RI * 128], IH1, tbw_h1,
                                  gBh1, tB_c[:, IH0 * 128:RI * 128, :], None)
                av = accA[:].rearrange("p (j f) -> p j f", j=RI)
                nc.vector.tensor_scalar(
                    out=accC[:].rearrange("p (j f) -> p j f", j=RI),
                    in0=av[:, :, 64:128],
                    scalar1=0.5, scalar2=None,
                    op0=mybir.AluOpType.mult)
                nc.vector.tensor_add(accC[:], accC[:], eif[:])

                def hookB0():
                    cast_write_gather(accB[:, :UH0 * 128], UH0, tcw_h0,
                                      gCh0, tC_c[:, 0:UH0 * 128, :],
                                      (1.0 / 3.0, 0.25))
                spmm(sl_B, 2, NWU, UH0, ui_nch, ui_idx, ui_S, accB,
                     False, hookB0)
                cast_write_gather(accB[:, UH0 * 128:RU * 128], UH1, tcw_h1,
                                  gCh1, tC_c[:, UH0 * 128:RU * 128, :],
                                  (1.0 / 3.0, 0.25))

                spmm(sl_C, 4, NWI, IH0, iu_nch, iu_idx, iu_S, accC,
                     True, None)

            nc.sync.dma_start(
                out=out_ext[0:48 * 128, :].rearrange("(j p) d -> p j d", p=128),
                in_=accC[:, :48 * 64].rearrange("p (j f) -> p j f", j=48))
            nc.sync.dma_start(
                out=out_ext[48 * 128:DI, :],
                in_=accC[0:DI - 48 * 128, 48 * 64:49 * 64])

    nc.compile()
    return nc


def _host_tables(embed_user):
    eu = np.asarray(embed_user, np.float32)
    eu_tbl = np.zeros((8 * RU * 128, 128), BF16)
    r = np.arange(U)
    c = r // DU
    rl = r % DU
    j, p = rl // 128, rl % 128
    pos = c * (RU * 128) + j * 128 + p
    eu_tbl[pos, 64:128] = eu.astype(BF16)
    return eu_tbl


def _make_in_maps(inputs, iu, ui):
    eu_tbl = _host_tables(inputs["embed_user"])
    ei = np.asarray(inputs["embed_item"], np.float32)
    in_maps = []
    for c in range(NCORES):
        ei_sl = ei[c * DI:(c + 1) * DI]
        eiw = np.zeros((128, RI * 128), BF16)
        eif = np.zeros((128, RI * 64), np.float32)
        r = np.arange(DI)
        p, j = r % 128, r // 128
        eiw_view = eiw.reshape(128, RI, 128)
        eif_view = eif.reshape(128, RI, 64)
        eiw_view[p, j, 0:64] = ei_sl.astype(BF16)
        eif_view[p, j, :] = ei_sl
        in_maps.append({
            "eu_tbl": eu_tbl,
            "ei_w": eiw,
            "ei_f32w": eif,
            "iu_idx": iu["idx16"][c], "iu_S": iu["S"][c],
            "ui_idx": ui["idx16"][c], "ui_S": ui["S"][c],
        })
    return in_maps


def kernel(embed_user, embed_item, edge_vals, u_idx, i_idx):
    global _LAST_RESULTS
    inputs = {
        "embed_user": np.asarray(embed_user, np.float32),
        "embed_item": np.asarray(embed_item, np.float32),
        "edge_vals": np.asarray(edge_vals, np.float32),
        "u_idx": u_idx, "i_idx": i_idx,
    }
    iu, ui = _pack_all(inputs)
    nc = _build_v2(iu["nch"], ui["nch"])
    in_maps = _make_in_maps(inputs, iu, ui)
    res = bass_utils.run_bass_kernel_spmd(
        nc, in_maps, core_ids=list(range(NCORES)),
        trace=bool(int(os.environ.get("KERNEL_TRACE", "0"))))
    _LAST_RESULTS = res
    out = np.concatenate([res.results[c]["out"] for c in range(NCORES)], axis=0)
    return out
